# revision 17
# baseline (speedup 1.0000x reference)
"""ViT attention block (B=8, N=1024, dim=1024, heads=16, d_k=64) on 8 trn2 NeuronCores.

Sharding: data-parallel over batch (1 batch per core), weights replicated.
No collectives; each core computes its batch's full attention output.

v2 design (exp-stream centric). Per-core:
  - Q/K projections run in fp8e4 DoubleRow (K=256 per matmul): host ships
    x and w_qkv[:, :2048] as fp8 "dim-pair" tensors [128, 2, *]; w scaled by
    32 (values would be subnormal in e4m3 otherwise), compensated by folding
    1/(32*32) into the exp scale. V projection stays bf16 (fp8 V costs too
    much accuracy).
  - S^T quarters: one matmul = [128 m, 512 n] for one (mt, nh, head); the
    two heads of a pair run as concurrent 64-row-group matmuls. Quarters
    stream into ping-pong PSUM tiles of 3 quarters ([128, 1536], 3 banks x
    2 bufs) so ScalarE's exp (the critical engine, ~1.9us per tile) never
    waits on a PSUM WAR hazard: S^T for tile t+1 fills while exp reads t.
  - exp(scale*S) out of PSUM -> et bf16 in SBUF (max-subtraction skipped:
    |scale*S| <~ 2, exp is exact-safe and softmax shift-invariant).
  - V' = x @ w_v with a constant-1 column per head (65 cols) so PV yields
    softmax row-sums for free; V' matmuls are emitted lazily inside the
    exp phase (PE slack) instead of a serial prologue.
  - PV(p) trails one pair behind the exp stream; [65,512] psum chains over
    8 m-tiles; staged to stg, denominator row reshaped via DRAM for a wide
    reciprocal, broadcast back, fused into the normalize multiply.
  - final = attnT.T @ w_out + b_out in the tail.
"""

import os
import numpy as np
import ml_dtypes

import concourse.bass as bass
from concourse import bacc
import concourse.mybir as mybir
import concourse.tile as tile
from concourse.bass_utils import run_bass_kernel_spmd

P = 128
N_TOK = 1024
DIM = 1024
HEADS = 16
D_K = 64
N_CORES = 8
SCALE = D_K ** -0.5  # 0.125

NP_T = N_TOK // P   # 8 token tiles
DP = DIM // P       # 8 dim tiles
KP = DP // 2        # 4 dim-pair tiles for fp8 DoubleRow
NPAIRS = HEADS // 2  # 8 head pairs
VW = D_K + 1        # 65: V columns per head incl. ones column
W8SCALE = 32.0      # host-side w_qkv fp8 pre-scale (both q and k cols)

NQ = NPAIRS * 32    # 256 S^T quarters ([128, 512] each)
TILE_Q = 3          # quarters per st/et tile
N_ST = (NQ + TILE_Q - 1) // TILE_Q  # 86 tiles (last holds 1 quarter)

BF16 = mybir.dt.bfloat16
F8 = mybir.dt.float8e4
F32 = mybir.dt.float32
DR = mybir.MatmulPerfMode.DoubleRow

# "fp8" (default) = Q/K projection in fp8 DoubleRow; "bf16" = all-bf16
QK_MODE = os.environ.get("KERNEL_QK_MODE", "fp8")


def build_program():
    nc = bacc.Bacc("TRN2", target_bir_lowering=False, debug=False)

    xT = nc.dram_tensor("xT", [DIM, N_TOK], BF16, kind="ExternalInput").ap()
    wv = nc.dram_tensor("w_v", [DIM, DIM], BF16, kind="ExternalInput").ap()
    wout = nc.dram_tensor("w_out", [DIM, DIM], BF16, kind="ExternalInput").ap()
    bout = nc.dram_tensor("b_out", [DIM], F32, kind="ExternalInput").ap()
    if QK_MODE == "fp8":
        x8 = nc.dram_tensor("x8", [KP, P, 2, N_TOK], F8,
                            kind="ExternalInput").ap()
        w8 = nc.dram_tensor("w8", [16, P, KP, 2, P], F8,
                            kind="ExternalInput").ap()
        exp_scale = float(SCALE) / (W8SCALE * W8SCALE)
    else:
        wqk = nc.dram_tensor("w_qk", [DIM, 2 * DIM], BF16,
                             kind="ExternalInput").ap()
        exp_scale = float(SCALE)
    out = nc.dram_tensor("out", [N_TOK, DIM], F32, kind="ExternalOutput").ap()
    rs_dram = nc.dram_tensor("rs_scratch", [HEADS, N_TOK], F32).ap()
    rs2_dram = nc.dram_tensor("rs2_scratch", [HEADS, N_TOK], F32).ap()
    DEBUG = os.environ.get("KERNEL_DEBUG", "0") == "1"
    if DEBUG:
        dbg_qkt = nc.dram_tensor("dbg_qkt", [2, P, N_TOK], BF16,
                                 kind="ExternalOutput").ap()
        dbg_et = nc.dram_tensor("dbg_et", [3, P, 1536], BF16,
                                kind="ExternalOutput").ap()
        dbg_v = nc.dram_tensor("dbg_v", [P, HEADS * VW], BF16,
                               kind="ExternalOutput").ap()
        dbg_attnT = nc.dram_tensor("dbg_attnT", [P, N_TOK], BF16,
                                   kind="ExternalOutput").ap()
        dbg_stg = nc.dram_tensor("dbg_stg", [2, VW, N_TOK], F32,
                                 kind="ExternalOutput").ap()
        dbg_rcp = nc.dram_tensor("dbg_rcp", [2, D_K, N_TOK], F32,
                                 kind="ExternalOutput").ap()

    with tile.TileContext(nc) as tc:
        with (
            tc.tile_pool(name="persist", bufs=1) as persist,
            tc.tile_pool(name="qkt", bufs=5) as qktp,
            tc.tile_pool(name="etp", bufs=22) as etp,
            tc.tile_pool(name="stg", bufs=3) as stgp,
            tc.tile_pool(name="small", bufs=2) as small,
            tc.tile_pool(name="w8p", bufs=4) as w8p,
        ):
            # ---------------- persistent SBUF ----------------
            x8_sb = []
            if QK_MODE == "fp8":
                for k2 in range(KP):
                    t = persist.tile([P, 2, N_TOK], F8, tag=f"x8_{k2}",
                                     name=f"x8_{k2}")
                    nc.sync.dma_start(t[:], x8[k2])
                    x8_sb.append(t)
            xT_sb = []
            wv_sb = []
            for k in range(DP):
                t = persist.tile([P, N_TOK], BF16, tag=f"xT{k}", name=f"xT{k}")
                xT_sb.append(t)
                w = persist.tile([P, DIM], BF16, tag=f"wv{k}", name=f"wv{k}")
                wv_sb.append(w)
            v_sb = []
            for j in range(NP_T):
                v_sb.append(persist.tile([P, HEADS * VW], BF16, tag=f"v{j}",
                                         name=f"v{j}"))
            attnT_sb = []
            for p in range(NPAIRS):
                attnT_sb.append(persist.tile([P, N_TOK], BF16, tag=f"attnT{p}",
                                             name=f"attnT{p}"))
            wout_sb = []
            for k in range(DP):
                w = persist.tile([P, DIM], BF16, tag=f"wout{k}",
                                 name=f"wout{k}")
                wout_sb.append(w)
            bias_bc = persist.tile([P, DIM], F32, tag="bias")
            bias_in = bass.AP(tensor=bout.tensor, offset=bout.offset,
                              ap=[[0, P]] + list(bout.ap))

            st_tiles = {}   # t -> PSUM tile (ping-pong)
            et_tiles = {}   # t -> SBUF bf16 tile
            qkt_done = {}   # ('q'|'k', pair) -> finished [128,1024] bf16 tile
            inflight = {}

            with (
                tc.tile_pool(name="ppv", bufs=1, space="PSUM") as ppv,
                tc.tile_pool(name="pq2", bufs=1, space="PSUM") as pq2,
            ):
                # ---------- QKT M-tile emission (chunk = one nh half) ----
                def qkt_chunk(which, pair, nh):
                    """Emit half of the QT/KT M-tile for `pair`. Returns the
                    finished [128,1024] bf16 tile after the 2nd chunk."""
                    key = (which, pair)
                    colbase = (0 if which == 'q' else DIM) + pair * P
                    if key not in inflight:
                        inflight[key] = qktp.tile([P, N_TOK], BF16, tag="qkt",
                                                  name=f"qkt_{which}{pair}")
                    dest = inflight[key]
                    ps = pq2.tile([P, 512], F32, tag="pq2",
                                  name=f"psqk_{which}{pair}_{nh}")
                    if QK_MODE == "fp8":
                        m_idx = (0 if which == 'q' else 8) + pair
                        wkey = ("w8t", which, pair)
                        if nh == 0:
                            wt = w8p.tile([P, KP, 2, P], F8, tag="w8",
                                          name=f"w8_{which}{pair}")
                            nc.sync.dma_start(wt[:], w8[m_idx])
                            inflight[wkey] = wt
                        wt = inflight[wkey]
                        for k2 in range(KP):
                            nc.tensor.matmul(
                                ps[:],
                                lhsT=wt[:, k2, :, :],
                                rhs=x8_sb[k2][:, :, nh * 512:(nh + 1) * 512],
                                start=(k2 == 0), stop=(k2 == KP - 1),
                                perf_mode=DR,
                            )
                        if nh == 1:
                            del inflight[wkey]
                    else:
                        for k in range(DP):
                            w = w8p.tile([P, P], BF16, tag="wqk",
                                         name=f"wqk_{which}{pair}_{nh}_{k}")
                            nc.sync.dma_start(
                                w[:], wqk[k * P:(k + 1) * P,
                                          colbase:colbase + P])
                            nc.tensor.matmul(
                                ps[:],
                                lhsT=w[:],
                                rhs=xT_sb[k][:, nh * 512:(nh + 1) * 512],
                                start=(k == 0), stop=(k == DP - 1),
                            )
                    nc.vector.tensor_copy(
                        out=dest[:, nh * 512:(nh + 1) * 512], in_=ps[:])
                    if nh == 1:
                        del inflight[key]
                        qkt_done[key] = dest
                        return dest
                    return None

                # ---------- V' chunk (j, nh): 8 bf16 matmuls ----------
                def v_chunk(j, nh):
                    if nh == 0:
                        nc.vector.memset(
                            v_sb[j][:].rearrange("p (h x) -> p h x",
                                                 x=VW)[:, :, D_K:], 1.0)
                    ps = pq2.tile([P, 512], F32, tag="pq2", name=f"psv{j}_{nh}")
                    for k in range(DP):
                        nc.tensor.matmul(
                            ps[:],
                            lhsT=xT_sb[k][:, j * P:(j + 1) * P],
                            rhs=wv_sb[k][:, nh * 512:(nh + 1) * 512],
                            start=(k == 0), stop=(k == DP - 1),
                        )
                    hs = nh * (HEADS // 2)
                    nc.vector.tensor_copy(
                        out=v_sb[j][:].rearrange(
                            "p (h x) -> p h x",
                            x=VW)[:, hs:hs + HEADS // 2, :D_K],
                        in_=ps[:].rearrange("p (h d) -> p h d", d=D_K),
                    )

                # ---------- S^T quarter stream ----------
                with tc.tile_pool(name="stp", bufs=2, space="PSUM") as stp:
                    def st_quarter(g, pair, mt, nh, h):
                        t, q = g // TILE_Q, g % TILE_Q
                        if q == 0:
                            width = min(TILE_Q, NQ - t * TILE_Q) * 512
                            st_tiles[t] = stp.tile([P, width], F32, tag="st",
                                                   name=f"st{t}")
                            et_tiles[t] = etp.tile([P, width], BF16, tag="et",
                                                   name=f"et{t}")
                        kt = qkt_done[('k', pair)]
                        qt = qkt_done[('q', pair)]
                        nc.tensor.matmul(
                            st_tiles[t][:, q * 512:(q + 1) * 512],
                            lhsT=kt[h * D_K:(h + 1) * D_K,
                                    mt * P:(mt + 1) * P],
                            rhs=qt[h * D_K:(h + 1) * D_K,
                                   nh * 512:(nh + 1) * 512],
                            start=True, stop=True,
                            tile_position=(h * D_K, 0),
                        )
                        if q == TILE_Q - 1 or g == NQ - 1:
                            nc.scalar.activation(
                                et_tiles[t][:], st_tiles[t][:],
                                mybir.ActivationFunctionType.Exp,
                                scale=exp_scale)
                            del st_tiles[t]
                            if DEBUG and t < 3:
                                nc.sync.dma_start(dbg_et[t], et_tiles[t][:])

                    def et_slice(pair, mt, nh, h):
                        g = 32 * pair + 4 * mt + 2 * nh + h
                        t, q = g // TILE_Q, g % TILE_Q
                        return et_tiles[t][:, q * 512:(q + 1) * 512]

                    # ---------- PV + normalize ----------
                    pending_norm = {}

                    def normalize_a(p, h, stg, last):
                        hg = 2 * p + h
                        dma = nc.sync.dma_start if last else \
                            nc.gpsimd.dma_start
                        dma(rs_dram[hg:hg + 1, :], stg[D_K:VW, :])
                        rsp = small.tile([P, NP_T], F32, tag="rsp",
                                         name=f"rsp{hg}")
                        dma(rsp[:], rs_dram[hg].rearrange("(p i) -> p i", p=P))
                        pending_norm[(p, h)] = (stg, rsp, last)

                    def normalize_b(p, h):
                        if (p, h) not in pending_norm:
                            return
                        stg, rsp, last = pending_norm.pop((p, h))
                        hg = 2 * p + h
                        dma = nc.sync.dma_start if last else \
                            nc.gpsimd.dma_start
                        rspr = small.tile([P, NP_T], F32, tag="rspr",
                                          name=f"rspr{hg}")
                        nc.vector.reciprocal(rspr[:], rsp[:])
                        dma(rs2_dram[hg].rearrange("(p i) -> p i", p=P),
                            rspr[:])
                        rs_row = rs2_dram[hg:hg + 1, :]
                        rs_bc = bass.AP(tensor=rs_row.tensor,
                                        offset=rs_row.offset,
                                        ap=[[0, D_K], list(rs_row.ap)[-1]])
                        rcp = small.tile([D_K, N_TOK], F32, tag="rcp",
                                         name=f"rcp{hg}")
                        dma(rcp[:], rs_bc)
                        if h == 0:
                            nc.vector.tensor_mul(out=attnT_sb[p][0:D_K, :],
                                                 in0=stg[0:D_K, :], in1=rcp[:])
                        else:
                            tmp = small.tile([D_K, N_TOK], BF16, tag="oddtmp",
                                             name=f"oddtmp{hg}")
                            nc.vector.tensor_mul(out=tmp[:],
                                                 in0=stg[0:D_K, :], in1=rcp[:])
                            dma(attnT_sb[p][D_K:P, :], tmp[:])

                    def normalize_evict(p, h, stg, last):
                        normalize_a(p, h, stg, last)
                        if last:
                            normalize_b(p, h)

                    def pv_chunk(p, slot16, last=False):
                        """4 slots per (h, nh) chain: 2 matmuls each."""
                        h, nh = slot16 // 8, (slot16 // 4) % 2
                        hg = 2 * p + h
                        q = slot16 % 4
                        if q == 0:
                            inflight[(p, h, nh)] = ppv.tile(
                                [VW, 512], F32, tag="ppv",
                                name=f"pv{p}_{h}_{nh}")
                        pvt = inflight[(p, h, nh)]
                        for mt in range(2 * q, 2 * q + 2):
                            nc.tensor.matmul(
                                pvt[:],
                                lhsT=v_sb[mt][:, hg * VW:(hg + 1) * VW],
                                rhs=et_slice(p, mt, nh, h),
                                start=(mt == 0), stop=(mt == NP_T - 1),
                            )
                        if q == 3:
                            if nh == 0:
                                inflight[("stg", p, h)] = stgp.tile(
                                    [VW, N_TOK], F32, tag="stg",
                                    name=f"stg{hg}")
                            stg = inflight[("stg", p, h)]
                            nc.vector.tensor_copy(
                                out=stg[:, nh * 512:(nh + 1) * 512],
                                in_=pvt[:])
                            del inflight[(p, h, nh)]
                            if nh == 1:
                                normalize_evict(p, h, stg, last)
                                del inflight[("stg", p, h)]
                                if h == 1:
                                    lo = 32 * p
                                    for t in range(lo // TILE_Q):
                                        et_tiles.pop(t, None)

                    # ================= ramp: pair-0 QT/KT =================
                    for nh in range(2):
                        qkt_chunk('q', 0, nh)
                    for nh in range(2):
                        qkt_chunk('k', 0, nh)

                    if DEBUG:
                        nc.sync.dma_start(dbg_qkt[0], qkt_done[('q', 0)][:])
                        nc.sync.dma_start(dbg_qkt[1], qkt_done[('k', 0)][:])

                    # deferred bulk DMAs: xT/wv on the gpsimd queue so the
                    # sync queue serves the ramp's w8 slices first
                    for k in range(DP):
                        nc.gpsimd.dma_start(xT_sb[k][:], xT[k * P:(k + 1) * P, :])
                        nc.gpsimd.dma_start(wv_sb[k][:], wv[k * P:(k + 1) * P, :])
                    for k in range(DP):
                        nc.sync.dma_start(wout_sb[k][:], wout[k * P:(k + 1) * P, :])
                    nc.sync.dma_start(bias_bc[:], bias_in)

                    # ================= main pair loop =================
                    for p in range(NPAIRS):
                        for s in range(16):      # slot = (mt, nh)
                            mt, nh = s // 2, s % 2
                            # filler: pair p+1 QT/KT (4 chunks per pair)
                            if p + 1 < NPAIRS and s < 8 and s % 2 == 0:
                                c = s // 2
                                qkt_chunk('q' if c < 2 else 'k', p + 1, c % 2)
                            # V' chunks during pair 0 (all before PV(0)),
                            # starting at slot 4 so the xT/wv DMAs can land
                            if p == 0 and s >= 4:
                                cs = [s - 4] if s < 12 else [8 + (s - 12) * 2, 9 + (s - 12) * 2]
                                for c in cs:
                                    v_chunk(c // 2, c % 2)
                            # S^T quarters (2 concurrent row-group matmuls)
                            for h in range(2):
                                g = 32 * p + 4 * mt + 2 * nh + h
                                st_quarter(g, p, mt, nh, h)
                            # PV for pair p-1 (4 slots per chain)
                            if p > 0:
                                pv_chunk(p - 1, s)
                            # deferred normalize (recip+muls) once DMAs landed
                            if s == 4 and p >= 2:
                                normalize_b(p - 2, 1)
                            if s == 12 and p >= 1:
                                normalize_b(p - 1, 0)

                if DEBUG:
                    nc.sync.dma_start(dbg_v, v_sb[0][:])
                    nc.sync.dma_start(dbg_attnT, attnT_sb[0][:])

                # ============== tail: PV(7) + projection ==============
                # (outside the stp scope so pproj's 4 banks fit)
                with (
                    tc.tile_pool(name="ev", bufs=4) as ev,
                    tc.tile_pool(name="pproj", bufs=6,
                                 space="PSUM") as pproj,
                ):
                    normalize_b(NPAIRS - 2, 1)

                    proj_ps = {}

                    def proj_mm(j, nh, p):
                        key = (j, nh)
                        if key not in proj_ps:
                            proj_ps[key] = pproj.tile(
                                [P, 512], F32, tag="pproj",
                                name=f"pso{j}_{nh}")
                        nc.tensor.matmul(
                            proj_ps[key][:],
                            lhsT=attnT_sb[p][:, j * P:(j + 1) * P],
                            rhs=wout_sb[p][:, nh * 512:(nh + 1) * 512],
                            start=(p == 0), stop=(p == NPAIRS - 1),
                        )

                    def proj_finish(j, nh):
                        ps = proj_ps.pop((j, nh))
                        o = ev.tile([P, 512], F32, tag="out",
                                    name=f"o{j}_{nh}")
                        nc.vector.tensor_add(
                            out=o[:], in0=ps[:],
                            in1=bias_bc[:, nh * 512:(nh + 1) * 512])
                        nc.sync.dma_start(
                            out[j * P:(j + 1) * P, nh * 512:(nh + 1) * 512],
                            o[:])

                    groups = [[(j, nh) for j in js for nh in range(2)]
                              for js in ((0, 1, 2), (3, 4, 5), (6, 7))]
                    # PV(7) even-head chains, then group-A projection pass 1
                    # (pairs 0-6, inputs ready) to cover normalize latency
                    for slot16 in range(8):
                        pv_chunk(NPAIRS - 1, slot16, last=True)
                    for c in groups[0]:
                        for p in range(NPAIRS - 1):
                            proj_mm(*c, p)
                    for slot16 in range(8, 16):
                        pv_chunk(NPAIRS - 1, slot16, last=True)
                    for c in groups[0]:
                        proj_mm(*c, NPAIRS - 1)
                    for c in groups[0]:
                        proj_finish(*c)
                    for grp in groups[1:]:
                        for c in grp:
                            for p in range(NPAIRS):
                                proj_mm(*c, p)
                        for c in grp:
                            proj_finish(*c)

    nc.compile()
    return nc


_NC_CACHE = None


def _get_program():
    global _NC_CACHE
    if _NC_CACHE is None:
        _NC_CACHE = build_program()
    return _NC_CACHE


def make_in_maps(x, w_qkv, w_out, b_out):
    F8NP = ml_dtypes.float8_e4m3fn
    w_qkv = np.ascontiguousarray(w_qkv).astype(np.float32)
    wv_c = np.ascontiguousarray(w_qkv[:, 2 * DIM:]).astype(ml_dtypes.bfloat16)
    w_out_c = np.ascontiguousarray(w_out).astype(ml_dtypes.bfloat16)
    b_out_c = np.ascontiguousarray(b_out).astype(np.float32)
    common = {
        "w_v": wv_c,
        "w_out": w_out_c,
        "b_out": b_out_c,
    }
    if QK_MODE == "fp8":
        # w8: [KP, 128, 2, 2048], plane i = dim-tile (2*k2 + i)
        wqk8 = (w_qkv[:, :2 * DIM] * W8SCALE).astype(F8NP)
        common["w8"] = np.ascontiguousarray(
            wqk8.reshape(KP, 2, P, 16, P).transpose(3, 2, 0, 1, 4))
    else:
        common["w_qk"] = np.ascontiguousarray(
            w_qkv[:, :2 * DIM]).astype(ml_dtypes.bfloat16)
    in_maps = []
    for b in range(N_CORES):
        xb = np.asarray(x[b], dtype=np.float32)
        xTb = np.ascontiguousarray(xb.T)
        m = dict(common)
        m["xT"] = xTb.astype(ml_dtypes.bfloat16)
        if QK_MODE == "fp8":
            x8b = xTb.astype(F8NP)  # [dim, tok]
            m["x8"] = np.ascontiguousarray(
                x8b.reshape(KP, 2, P, N_TOK).transpose(0, 2, 1, 3))
        in_maps.append(m)
    return in_maps


def kernel(x, w_qkv, w_out, b_out):
    nc = _get_program()
    in_maps = make_in_maps(x, w_qkv, w_out, b_out)
    res = run_bass_kernel_spmd(nc, in_maps, list(range(N_CORES)))
    outs = [np.asarray(r["out"], dtype=np.float32) for r in res.results]
    return np.stack(outs, axis=0)


# revision 18
# speedup vs baseline: 1.1407x; 1.1407x over previous
"""ViT attention block (B=8, N=1024, dim=1024, heads=16, d_k=64) on 8 trn2 NeuronCores.

Sharding: data-parallel over batch (1 batch per core), weights replicated.
No collectives; each core computes its batch's full attention output.

v2 design (exp-stream centric). Per-core:
  - Q/K projections run in fp8e4 DoubleRow (K=256 per matmul): host ships
    x and w_qkv[:, :2048] as fp8 "dim-pair" tensors [128, 2, *]; w scaled by
    32 (values would be subnormal in e4m3 otherwise), compensated by folding
    1/(32*32) into the exp scale. V projection stays bf16 (fp8 V costs too
    much accuracy).
  - S^T quarters: one matmul = [128 m, 512 n] for one (mt, nh, head); the
    two heads of a pair run as concurrent 64-row-group matmuls. Quarters
    stream into ping-pong PSUM tiles of 3 quarters ([128, 1536], 3 banks x
    2 bufs) so ScalarE's exp (the critical engine, ~1.9us per tile) never
    waits on a PSUM WAR hazard: S^T for tile t+1 fills while exp reads t.
  - exp(scale*S) out of PSUM -> et bf16 in SBUF (max-subtraction skipped:
    |scale*S| <~ 2, exp is exact-safe and softmax shift-invariant).
  - V' = x @ w_v with a constant-1 column per head (65 cols) so PV yields
    softmax row-sums for free; V' matmuls are emitted lazily inside the
    exp phase (PE slack) instead of a serial prologue.
  - PV(p) trails one pair behind the exp stream; [65,512] psum chains over
    8 m-tiles; staged to stg, denominator row reshaped via DRAM for a wide
    reciprocal, broadcast back, fused into the normalize multiply.
  - final = attnT.T @ w_out + b_out in the tail.
"""

import os
import numpy as np
import ml_dtypes

import concourse.bass as bass
from concourse import bacc
import concourse.mybir as mybir
import concourse.tile as tile
from concourse.bass_utils import run_bass_kernel_spmd

P = 128
N_TOK = 1024
DIM = 1024
HEADS = 16
D_K = 64
N_CORES = 8
SCALE = D_K ** -0.5  # 0.125

NP_T = N_TOK // P   # 8 token tiles
DP = DIM // P       # 8 dim tiles
KP = DP // 2        # 4 dim-pair tiles for fp8 DoubleRow
NPAIRS = HEADS // 2  # 8 head pairs
VW = D_K + 1        # 65: V columns per head incl. ones column
W8SCALE = 32.0      # host-side w_qkv fp8 pre-scale (both q and k cols)

NQ = NPAIRS * 32    # 256 S^T quarters ([128, 512] each)
TILE_Q = 3          # quarters per st/et tile
N_ST = (NQ + TILE_Q - 1) // TILE_Q  # 86 tiles (last holds 1 quarter)

BF16 = mybir.dt.bfloat16
F8 = mybir.dt.float8e4
F32 = mybir.dt.float32
DR = mybir.MatmulPerfMode.DoubleRow

# "fp8" (default) = Q/K projection in fp8 DoubleRow; "bf16" = all-bf16
QK_MODE = os.environ.get("KERNEL_QK_MODE", "fp8")


def build_program():
    nc = bacc.Bacc("TRN2", target_bir_lowering=False, debug=False)

    xT = nc.dram_tensor("xT", [DIM, N_TOK], BF16, kind="ExternalInput").ap()
    wv = nc.dram_tensor("w_v", [DIM, DIM], BF16, kind="ExternalInput").ap()
    wout = nc.dram_tensor("w_out", [DIM, DIM], BF16, kind="ExternalInput").ap()
    bout = nc.dram_tensor("b_out", [DIM], F32, kind="ExternalInput").ap()
    if QK_MODE == "fp8":
        x8 = nc.dram_tensor("x8", [KP, P, 2, N_TOK], F8,
                            kind="ExternalInput").ap()
        w8 = nc.dram_tensor("w8", [16, P, KP, 2, P], F8,
                            kind="ExternalInput").ap()
        exp_scale = float(SCALE) / (W8SCALE * W8SCALE)
    else:
        wqk = nc.dram_tensor("w_qk", [DIM, 2 * DIM], BF16,
                             kind="ExternalInput").ap()
        exp_scale = float(SCALE)
    out = nc.dram_tensor("out", [N_TOK, DIM], F32, kind="ExternalOutput").ap()
    rs_dram = nc.dram_tensor("rs_scratch", [HEADS, N_TOK], F32).ap()
    rs2_dram = nc.dram_tensor("rs2_scratch", [HEADS, N_TOK], F32).ap()
    DEBUG = os.environ.get("KERNEL_DEBUG", "0") == "1"
    if DEBUG:
        dbg_qkt = nc.dram_tensor("dbg_qkt", [2, P, N_TOK], BF16,
                                 kind="ExternalOutput").ap()
        dbg_et = nc.dram_tensor("dbg_et", [3, P, 1536], BF16,
                                kind="ExternalOutput").ap()
        dbg_v = nc.dram_tensor("dbg_v", [P, HEADS * VW], BF16,
                               kind="ExternalOutput").ap()
        dbg_attnT = nc.dram_tensor("dbg_attnT", [P, N_TOK], BF16,
                                   kind="ExternalOutput").ap()
        dbg_stg = nc.dram_tensor("dbg_stg", [2, VW, N_TOK], F32,
                                 kind="ExternalOutput").ap()
        dbg_rcp = nc.dram_tensor("dbg_rcp", [2, D_K, N_TOK], F32,
                                 kind="ExternalOutput").ap()

    with tile.TileContext(nc) as tc:
        with (
            tc.tile_pool(name="persist", bufs=1) as persist,
            tc.tile_pool(name="qkt", bufs=5) as qktp,
            tc.tile_pool(name="etp", bufs=22) as etp,
            tc.tile_pool(name="stg", bufs=3) as stgp,
            tc.tile_pool(name="small", bufs=2) as small,
            tc.tile_pool(name="w8p", bufs=4) as w8p,
        ):
            # ---------------- persistent SBUF ----------------
            x8_sb = []
            if QK_MODE == "fp8":
                for k2 in range(KP):
                    t = persist.tile([P, 2, N_TOK], F8, tag=f"x8_{k2}",
                                     name=f"x8_{k2}")
                    nc.sync.dma_start(t[:], x8[k2])
                    x8_sb.append(t)
            xT_sb = []
            wv_sb = []
            for k in range(DP):
                t = persist.tile([P, N_TOK], BF16, tag=f"xT{k}", name=f"xT{k}")
                xT_sb.append(t)
                w = persist.tile([P, DIM], BF16, tag=f"wv{k}", name=f"wv{k}")
                wv_sb.append(w)
            v_sb = []
            for j in range(NP_T):
                v_sb.append(persist.tile([P, HEADS * VW], BF16, tag=f"v{j}",
                                         name=f"v{j}"))
            attnT_sb = []
            for p in range(NPAIRS):
                attnT_sb.append(persist.tile([P, N_TOK], BF16, tag=f"attnT{p}",
                                             name=f"attnT{p}"))
            wout_sb = []
            for k in range(DP):
                w = persist.tile([P, DIM], BF16, tag=f"wout{k}",
                                 name=f"wout{k}")
                wout_sb.append(w)
            bias_bc = persist.tile([P, DIM], F32, tag="bias")
            bias_in = bass.AP(tensor=bout.tensor, offset=bout.offset,
                              ap=[[0, P]] + list(bout.ap))

            st_tiles = {}   # t -> PSUM tile (ping-pong)
            et_tiles = {}   # t -> SBUF bf16 tile
            qkt_done = {}   # ('q'|'k', pair) -> finished [128,1024] bf16 tile
            inflight = {}

            with (
                tc.tile_pool(name="ppv", bufs=1, space="PSUM") as ppv,
                tc.tile_pool(name="pq2", bufs=1, space="PSUM") as pq2,
            ):
                # ---------- QKT M-tile emission (chunk = one nh half) ----
                def qkt_chunk(which, pair, nh):
                    """Emit half of the QT/KT M-tile for `pair`. Returns the
                    finished [128,1024] bf16 tile after the 2nd chunk."""
                    key = (which, pair)
                    colbase = (0 if which == 'q' else DIM) + pair * P
                    if key not in inflight:
                        inflight[key] = qktp.tile([P, N_TOK], BF16, tag="qkt",
                                                  name=f"qkt_{which}{pair}")
                    dest = inflight[key]
                    ps = pq2.tile([P, 512], F32, tag="pq2",
                                  name=f"psqk_{which}{pair}_{nh}")
                    if QK_MODE == "fp8":
                        m_idx = (0 if which == 'q' else 8) + pair
                        wkey = ("w8t", which, pair)
                        if nh == 0:
                            wt = w8p.tile([P, KP, 2, P], F8, tag="w8",
                                          name=f"w8_{which}{pair}")
                            nc.sync.dma_start(wt[:], w8[m_idx])
                            inflight[wkey] = wt
                        wt = inflight[wkey]
                        for k2 in range(KP):
                            nc.tensor.matmul(
                                ps[:],
                                lhsT=wt[:, k2, :, :],
                                rhs=x8_sb[k2][:, :, nh * 512:(nh + 1) * 512],
                                start=(k2 == 0), stop=(k2 == KP - 1),
                                perf_mode=DR,
                            )
                        if nh == 1:
                            del inflight[wkey]
                    else:
                        for k in range(DP):
                            w = w8p.tile([P, P], BF16, tag="wqk",
                                         name=f"wqk_{which}{pair}_{nh}_{k}")
                            nc.sync.dma_start(
                                w[:], wqk[k * P:(k + 1) * P,
                                          colbase:colbase + P])
                            nc.tensor.matmul(
                                ps[:],
                                lhsT=w[:],
                                rhs=xT_sb[k][:, nh * 512:(nh + 1) * 512],
                                start=(k == 0), stop=(k == DP - 1),
                            )
                    nc.vector.tensor_copy(
                        out=dest[:, nh * 512:(nh + 1) * 512], in_=ps[:])
                    if nh == 1:
                        del inflight[key]
                        qkt_done[key] = dest
                        return dest
                    return None

                # ---------- V' chunk (j, nh): 8 bf16 matmuls ----------
                def v_chunk(j, nh):
                    if nh == 0:
                        nc.vector.memset(
                            v_sb[j][:].rearrange("p (h x) -> p h x",
                                                 x=VW)[:, :, D_K:], 1.0)
                    ps = pq2.tile([P, 512], F32, tag="pq2", name=f"psv{j}_{nh}")
                    for k in range(DP):
                        nc.tensor.matmul(
                            ps[:],
                            lhsT=xT_sb[k][:, j * P:(j + 1) * P],
                            rhs=wv_sb[k][:, nh * 512:(nh + 1) * 512],
                            start=(k == 0), stop=(k == DP - 1),
                        )
                    hs = nh * (HEADS // 2)
                    nc.vector.tensor_copy(
                        out=v_sb[j][:].rearrange(
                            "p (h x) -> p h x",
                            x=VW)[:, hs:hs + HEADS // 2, :D_K],
                        in_=ps[:].rearrange("p (h d) -> p h d", d=D_K),
                    )

                # ---------- S^T quarter stream ----------
                with tc.tile_pool(name="stp", bufs=2, space="PSUM") as stp:
                    def st_quarter(g, pair, mt, nh, h):
                        t, q = g // TILE_Q, g % TILE_Q
                        if q == 0:
                            width = min(TILE_Q, NQ - t * TILE_Q) * 512
                            st_tiles[t] = stp.tile([P, width], F32, tag="st",
                                                   name=f"st{t}")
                            et_tiles[t] = etp.tile([P, width], BF16, tag="et",
                                                   name=f"et{t}")
                        kt = qkt_done[('k', pair)]
                        qt = qkt_done[('q', pair)]
                        nc.tensor.matmul(
                            st_tiles[t][:, q * 512:(q + 1) * 512],
                            lhsT=kt[h * D_K:(h + 1) * D_K,
                                    mt * P:(mt + 1) * P],
                            rhs=qt[h * D_K:(h + 1) * D_K,
                                   nh * 512:(nh + 1) * 512],
                            start=True, stop=True,
                            tile_position=(h * D_K, 0),
                        )
                        if q == TILE_Q - 1 or g == NQ - 1:
                            nc.scalar.activation(
                                et_tiles[t][:], st_tiles[t][:],
                                mybir.ActivationFunctionType.Exp,
                                scale=exp_scale)
                            del st_tiles[t]
                            if DEBUG and t < 3:
                                nc.sync.dma_start(dbg_et[t], et_tiles[t][:])

                    def et_slice(pair, mt, nh, h):
                        g = 32 * pair + 4 * mt + 2 * nh + h
                        t, q = g // TILE_Q, g % TILE_Q
                        return et_tiles[t][:, q * 512:(q + 1) * 512]

                    # ---------- PV + normalize ----------
                    pending_norm = {}

                    def normalize_a(p, h, stg, last):
                        hg = 2 * p + h
                        dma = nc.sync.dma_start if last else \
                            nc.gpsimd.dma_start
                        dma(rs_dram[hg:hg + 1, :], stg[D_K:VW, :])
                        rsp = small.tile([P, NP_T], F32, tag="rsp",
                                         name=f"rsp{hg}")
                        dma(rsp[:], rs_dram[hg].rearrange("(p i) -> p i", p=P))
                        pending_norm[(p, h)] = (stg, rsp, last)

                    def normalize_b(p, h):
                        if (p, h) not in pending_norm:
                            return
                        stg, rsp, last = pending_norm.pop((p, h))
                        hg = 2 * p + h
                        dma = nc.sync.dma_start if last else \
                            nc.gpsimd.dma_start
                        rspr = small.tile([P, NP_T], F32, tag="rspr",
                                          name=f"rspr{hg}")
                        nc.vector.reciprocal(rspr[:], rsp[:])
                        dma(rs2_dram[hg].rearrange("(p i) -> p i", p=P),
                            rspr[:])
                        rs_row = rs2_dram[hg:hg + 1, :]
                        rs_bc = bass.AP(tensor=rs_row.tensor,
                                        offset=rs_row.offset,
                                        ap=[[0, D_K], list(rs_row.ap)[-1]])
                        rcp = small.tile([D_K, N_TOK], F32, tag="rcp",
                                         name=f"rcp{hg}")
                        dma(rcp[:], rs_bc)
                        if h == 0:
                            nc.vector.tensor_mul(out=attnT_sb[p][0:D_K, :],
                                                 in0=stg[0:D_K, :], in1=rcp[:])
                        else:
                            tmp = small.tile([D_K, N_TOK], BF16, tag="oddtmp",
                                             name=f"oddtmp{hg}")
                            nc.vector.tensor_mul(out=tmp[:],
                                                 in0=stg[0:D_K, :], in1=rcp[:])
                            dma(attnT_sb[p][D_K:P, :], tmp[:])

                    def normalize_evict(p, h, stg, last):
                        normalize_a(p, h, stg, last)
                        if last:
                            normalize_b(p, h)

                    def pv_chunk(p, slot16, last=False):
                        """4 slots per (h, nh) chain: 2 matmuls each."""
                        h, nh = slot16 // 8, (slot16 // 4) % 2
                        hg = 2 * p + h
                        q = slot16 % 4
                        if q == 0:
                            inflight[(p, h, nh)] = ppv.tile(
                                [VW, 512], F32, tag="ppv",
                                name=f"pv{p}_{h}_{nh}")
                        pvt = inflight[(p, h, nh)]
                        for mt in range(2 * q, 2 * q + 2):
                            nc.tensor.matmul(
                                pvt[:],
                                lhsT=v_sb[mt][:, hg * VW:(hg + 1) * VW],
                                rhs=et_slice(p, mt, nh, h),
                                start=(mt == 0), stop=(mt == NP_T - 1),
                            )
                        if q == 3:
                            if nh == 0:
                                inflight[("stg", p, h)] = stgp.tile(
                                    [VW, N_TOK], F32, tag="stg",
                                    name=f"stg{hg}")
                            stg = inflight[("stg", p, h)]
                            nc.vector.tensor_copy(
                                out=stg[:, nh * 512:(nh + 1) * 512],
                                in_=pvt[:])
                            del inflight[(p, h, nh)]
                            if nh == 1:
                                normalize_evict(p, h, stg, last)
                                del inflight[("stg", p, h)]
                                if h == 1:
                                    lo = 32 * p
                                    for t in range(lo // TILE_Q):
                                        et_tiles.pop(t, None)

                    # ================= ramp: pair-0 QT/KT =================
                    for nh in range(2):
                        qkt_chunk('q', 0, nh)
                    for nh in range(2):
                        qkt_chunk('k', 0, nh)

                    if DEBUG:
                        nc.sync.dma_start(dbg_qkt[0], qkt_done[('q', 0)][:])
                        nc.sync.dma_start(dbg_qkt[1], qkt_done[('k', 0)][:])

                    # deferred bulk DMAs: xT/wv on the gpsimd queue so the
                    # sync queue serves the ramp's w8 slices first
                    for k in range(DP):
                        nc.gpsimd.dma_start(xT_sb[k][:], xT[k * P:(k + 1) * P, :])
                        nc.gpsimd.dma_start(wv_sb[k][:], wv[k * P:(k + 1) * P, :])
                    for k in range(DP):
                        nc.sync.dma_start(wout_sb[k][:], wout[k * P:(k + 1) * P, :])
                    nc.sync.dma_start(bias_bc[:], bias_in)

                    # ================= main pair loop =================
                    for p in range(NPAIRS):
                        for s in range(16):      # slot = (mt, nh)
                            mt, nh = s // 2, s % 2
                            # filler: pair p+1 QT/KT (4 chunks per pair)
                            if p + 1 < NPAIRS and s < 8 and s % 2 == 0:
                                c = s // 2
                                qkt_chunk('q' if c < 2 else 'k', p + 1, c % 2)
                            # V' chunks during pair 0 (all before PV(0)),
                            # starting at slot 4 so the xT/wv DMAs can land
                            if p == 0 and s >= 4:
                                cs = [s - 4] if s < 12 else [8 + (s - 12) * 2, 9 + (s - 12) * 2]
                                for c in cs:
                                    v_chunk(c // 2, c % 2)
                            # S^T quarters (2 concurrent row-group matmuls)
                            for h in range(2):
                                g = 32 * p + 4 * mt + 2 * nh + h
                                st_quarter(g, p, mt, nh, h)
                            # PV for pair p-1 (4 slots per chain)
                            if p > 0:
                                pv_chunk(p - 1, s)
                            # deferred normalize (recip+muls) once DMAs landed
                            if s == 4 and p >= 2:
                                normalize_b(p - 2, 1)
                            if s == 12 and p >= 1:
                                normalize_b(p - 1, 0)

                if DEBUG:
                    nc.sync.dma_start(dbg_v, v_sb[0][:])
                    nc.sync.dma_start(dbg_attnT, attnT_sb[0][:])

                # ============== tail: PV(7) + projection ==============
                # (outside the stp scope so pproj's 4 banks fit)
                with (
                    tc.tile_pool(name="ev", bufs=2) as ev,
                    tc.tile_pool(name="pproj", bufs=2,
                                 space="PSUM") as pproj,
                ):
                    normalize_b(NPAIRS - 2, 1)
                    for slot16 in range(16):
                        pv_chunk(NPAIRS - 1, slot16, last=True)
                    for jg in range(NP_T // 2):
                        pss = []
                        for dj in range(2):
                            pss.append(pproj.tile([P, DIM], F32, tag="pproj",
                                                  name=f"pso{2 * jg + dj}"))
                        # pairs 0..6 for both j-tiles first (flow while the
                        # last pair's normalize is still in flight)
                        for p in range(NPAIRS):
                            for dj in range(2):
                                j = 2 * jg + dj
                                for nh in range(2):
                                    nc.tensor.matmul(
                                        pss[dj][:, nh * 512:(nh + 1) * 512],
                                        lhsT=attnT_sb[p][:, j * P:(j + 1) * P],
                                        rhs=wout_sb[p][:,
                                                       nh * 512:(nh + 1) * 512],
                                        start=(p == 0), stop=(p == NPAIRS - 1),
                                    )
                        for dj in range(2):
                            j = 2 * jg + dj
                            o = ev.tile([P, DIM], F32, tag="out", name=f"o{j}")
                            nc.vector.tensor_add(out=o[:], in0=pss[dj][:],
                                                 in1=bias_bc[:])
                            nc.sync.dma_start(out[j * P:(j + 1) * P, :], o[:])

    nc.compile()
    return nc


_NC_CACHE = None


def _get_program():
    global _NC_CACHE
    if _NC_CACHE is None:
        _NC_CACHE = build_program()
    return _NC_CACHE


def make_in_maps(x, w_qkv, w_out, b_out):
    F8NP = ml_dtypes.float8_e4m3fn
    w_qkv = np.ascontiguousarray(w_qkv).astype(np.float32)
    wv_c = np.ascontiguousarray(w_qkv[:, 2 * DIM:]).astype(ml_dtypes.bfloat16)
    w_out_c = np.ascontiguousarray(w_out).astype(ml_dtypes.bfloat16)
    b_out_c = np.ascontiguousarray(b_out).astype(np.float32)
    common = {
        "w_v": wv_c,
        "w_out": w_out_c,
        "b_out": b_out_c,
    }
    if QK_MODE == "fp8":
        # w8: [KP, 128, 2, 2048], plane i = dim-tile (2*k2 + i)
        wqk8 = (w_qkv[:, :2 * DIM] * W8SCALE).astype(F8NP)
        common["w8"] = np.ascontiguousarray(
            wqk8.reshape(KP, 2, P, 16, P).transpose(3, 2, 0, 1, 4))
    else:
        common["w_qk"] = np.ascontiguousarray(
            w_qkv[:, :2 * DIM]).astype(ml_dtypes.bfloat16)
    in_maps = []
    for b in range(N_CORES):
        xb = np.asarray(x[b], dtype=np.float32)
        xTb = np.ascontiguousarray(xb.T)
        m = dict(common)
        m["xT"] = xTb.astype(ml_dtypes.bfloat16)
        if QK_MODE == "fp8":
            x8b = xTb.astype(F8NP)  # [dim, tok]
            m["x8"] = np.ascontiguousarray(
                x8b.reshape(KP, 2, P, N_TOK).transpose(0, 2, 1, 3))
        in_maps.append(m)
    return in_maps


def kernel(x, w_qkv, w_out, b_out):
    nc = _get_program()
    in_maps = make_in_maps(x, w_qkv, w_out, b_out)
    res = run_bass_kernel_spmd(nc, in_maps, list(range(N_CORES)))
    outs = [np.asarray(r["out"], dtype=np.float32) for r in res.results]
    return np.stack(outs, axis=0)


# revision 19
# speedup vs baseline: 1.1507x; 1.0088x over previous
"""ViT attention block (B=8, N=1024, dim=1024, heads=16, d_k=64) on 8 trn2 NeuronCores.

Sharding: data-parallel over batch (1 batch per core), weights replicated.
No collectives; each core computes its batch's full attention output.

v2 design (exp-stream centric). Per-core:
  - Q/K projections run in fp8e4 DoubleRow (K=256 per matmul): host ships
    x and w_qkv[:, :2048] as fp8 "dim-pair" tensors [128, 2, *]; w scaled by
    32 (values would be subnormal in e4m3 otherwise), compensated by folding
    1/(32*32) into the exp scale. V projection stays bf16 (fp8 V costs too
    much accuracy).
  - S^T quarters: one matmul = [128 m, 512 n] for one (mt, nh, head); the
    two heads of a pair run as concurrent 64-row-group matmuls. Quarters
    stream into ping-pong PSUM tiles of 3 quarters ([128, 1536], 3 banks x
    2 bufs) so ScalarE's exp (the critical engine, ~1.9us per tile) never
    waits on a PSUM WAR hazard: S^T for tile t+1 fills while exp reads t.
  - exp(scale*S) out of PSUM -> et bf16 in SBUF (max-subtraction skipped:
    |scale*S| <~ 2, exp is exact-safe and softmax shift-invariant).
  - V' = x @ w_v with a constant-1 column per head (65 cols) so PV yields
    softmax row-sums for free; V' matmuls are emitted lazily inside the
    exp phase (PE slack) instead of a serial prologue.
  - PV(p) trails one pair behind the exp stream; [65,512] psum chains over
    8 m-tiles; staged to stg, denominator row reshaped via DRAM for a wide
    reciprocal, broadcast back, fused into the normalize multiply.
  - final = attnT.T @ w_out + b_out in the tail.
"""

import os
import numpy as np
import ml_dtypes

import concourse.bass as bass
from concourse import bacc
import concourse.mybir as mybir
import concourse.tile as tile
from concourse.bass_utils import run_bass_kernel_spmd

P = 128
N_TOK = 1024
DIM = 1024
HEADS = 16
D_K = 64
N_CORES = 8
SCALE = D_K ** -0.5  # 0.125

NP_T = N_TOK // P   # 8 token tiles
DP = DIM // P       # 8 dim tiles
KP = DP // 2        # 4 dim-pair tiles for fp8 DoubleRow
NPAIRS = HEADS // 2  # 8 head pairs
VW = D_K + 1        # 65: V columns per head incl. ones column
W8SCALE = 32.0      # host-side w_qkv fp8 pre-scale (both q and k cols)

NQ = NPAIRS * 32    # 256 S^T quarters ([128, 512] each)
TILE_Q = 3          # quarters per st/et tile
N_ST = (NQ + TILE_Q - 1) // TILE_Q  # 86 tiles (last holds 1 quarter)

BF16 = mybir.dt.bfloat16
F8 = mybir.dt.float8e4
F32 = mybir.dt.float32
DR = mybir.MatmulPerfMode.DoubleRow

# "fp8" (default) = Q/K projection in fp8 DoubleRow; "bf16" = all-bf16
QK_MODE = os.environ.get("KERNEL_QK_MODE", "fp8")


def build_program():
    nc = bacc.Bacc("TRN2", target_bir_lowering=False, debug=False)

    xT = nc.dram_tensor("xT", [DIM, N_TOK], BF16, kind="ExternalInput").ap()
    wv = nc.dram_tensor("w_v", [DIM, DIM], BF16, kind="ExternalInput").ap()
    wout = nc.dram_tensor("w_out", [DIM, DIM], BF16, kind="ExternalInput").ap()
    bout = nc.dram_tensor("b_out", [DIM], F32, kind="ExternalInput").ap()
    if QK_MODE == "fp8":
        x8 = nc.dram_tensor("x8", [KP, P, 2, N_TOK], F8,
                            kind="ExternalInput").ap()
        w8 = nc.dram_tensor("w8", [16, P, KP, 2, P], F8,
                            kind="ExternalInput").ap()
        exp_scale = float(SCALE) / (W8SCALE * W8SCALE)
    else:
        wqk = nc.dram_tensor("w_qk", [DIM, 2 * DIM], BF16,
                             kind="ExternalInput").ap()
        exp_scale = float(SCALE)
    out = nc.dram_tensor("out", [N_TOK, DIM], F32, kind="ExternalOutput").ap()
    rs_dram = nc.dram_tensor("rs_scratch", [HEADS, N_TOK], F32).ap()
    rs2_dram = nc.dram_tensor("rs2_scratch", [HEADS, N_TOK], F32).ap()
    DEBUG = os.environ.get("KERNEL_DEBUG", "0") == "1"
    if DEBUG:
        dbg_qkt = nc.dram_tensor("dbg_qkt", [2, P, N_TOK], BF16,
                                 kind="ExternalOutput").ap()
        dbg_et = nc.dram_tensor("dbg_et", [3, P, 1536], BF16,
                                kind="ExternalOutput").ap()
        dbg_v = nc.dram_tensor("dbg_v", [P, HEADS * VW], BF16,
                               kind="ExternalOutput").ap()
        dbg_attnT = nc.dram_tensor("dbg_attnT", [P, N_TOK], BF16,
                                   kind="ExternalOutput").ap()
        dbg_stg = nc.dram_tensor("dbg_stg", [2, VW, N_TOK], F32,
                                 kind="ExternalOutput").ap()
        dbg_rcp = nc.dram_tensor("dbg_rcp", [2, D_K, N_TOK], F32,
                                 kind="ExternalOutput").ap()

    with tile.TileContext(nc) as tc:
        with (
            tc.tile_pool(name="persist", bufs=1) as persist,
            tc.tile_pool(name="qkt", bufs=5) as qktp,
            tc.tile_pool(name="etp", bufs=22) as etp,
            tc.tile_pool(name="stg", bufs=3) as stgp,
            tc.tile_pool(name="small", bufs=2) as small,
            tc.tile_pool(name="w8p", bufs=4) as w8p,
        ):
            # ---------------- persistent SBUF ----------------
            x8_sb = []
            if QK_MODE == "fp8":
                for k2 in range(KP):
                    t = persist.tile([P, 2, N_TOK], F8, tag=f"x8_{k2}",
                                     name=f"x8_{k2}")
                    nc.sync.dma_start(t[:], x8[k2])
                    x8_sb.append(t)
            xT_sb = []
            wv_sb = []
            for k in range(DP):
                t = persist.tile([P, N_TOK], BF16, tag=f"xT{k}", name=f"xT{k}")
                xT_sb.append(t)
                w = persist.tile([P, DIM], BF16, tag=f"wv{k}", name=f"wv{k}")
                wv_sb.append(w)
            v_sb = []
            for j in range(NP_T):
                v_sb.append(persist.tile([P, HEADS * VW], BF16, tag=f"v{j}",
                                         name=f"v{j}"))
            attnT_sb = []
            for p in range(NPAIRS):
                attnT_sb.append(persist.tile([P, N_TOK], BF16, tag=f"attnT{p}",
                                             name=f"attnT{p}"))
            wout_sb = []
            for k in range(DP):
                w = persist.tile([P, DIM], BF16, tag=f"wout{k}",
                                 name=f"wout{k}")
                wout_sb.append(w)
            bias_bc = persist.tile([P, DIM], F32, tag="bias")
            bias_in = bass.AP(tensor=bout.tensor, offset=bout.offset,
                              ap=[[0, P]] + list(bout.ap))

            st_tiles = {}   # t -> PSUM tile (ping-pong)
            et_tiles = {}   # t -> SBUF bf16 tile
            qkt_done = {}   # ('q'|'k', pair) -> finished [128,1024] bf16 tile
            inflight = {}

            with (
                tc.tile_pool(name="ppv", bufs=1, space="PSUM") as ppv,
                tc.tile_pool(name="pq2", bufs=1, space="PSUM") as pq2,
            ):
                # ---------- QKT M-tile emission (chunk = one nh half) ----
                def qkt_chunk(which, pair, nh):
                    """Emit half of the QT/KT M-tile for `pair`. Returns the
                    finished [128,1024] bf16 tile after the 2nd chunk."""
                    key = (which, pair)
                    colbase = (0 if which == 'q' else DIM) + pair * P
                    if key not in inflight:
                        inflight[key] = qktp.tile([P, N_TOK], BF16, tag="qkt",
                                                  name=f"qkt_{which}{pair}")
                    dest = inflight[key]
                    ps = pq2.tile([P, 512], F32, tag="pq2",
                                  name=f"psqk_{which}{pair}_{nh}")
                    if QK_MODE == "fp8":
                        m_idx = (0 if which == 'q' else 8) + pair
                        wkey = ("w8t", which, pair)
                        if nh == 0:
                            wt = w8p.tile([P, KP, 2, P], F8, tag="w8",
                                          name=f"w8_{which}{pair}")
                            nc.sync.dma_start(wt[:], w8[m_idx])
                            inflight[wkey] = wt
                        wt = inflight[wkey]
                        for k2 in range(KP):
                            nc.tensor.matmul(
                                ps[:],
                                lhsT=wt[:, k2, :, :],
                                rhs=x8_sb[k2][:, :, nh * 512:(nh + 1) * 512],
                                start=(k2 == 0), stop=(k2 == KP - 1),
                                perf_mode=DR,
                            )
                        if nh == 1:
                            del inflight[wkey]
                    else:
                        for k in range(DP):
                            w = w8p.tile([P, P], BF16, tag="wqk",
                                         name=f"wqk_{which}{pair}_{nh}_{k}")
                            nc.sync.dma_start(
                                w[:], wqk[k * P:(k + 1) * P,
                                          colbase:colbase + P])
                            nc.tensor.matmul(
                                ps[:],
                                lhsT=w[:],
                                rhs=xT_sb[k][:, nh * 512:(nh + 1) * 512],
                                start=(k == 0), stop=(k == DP - 1),
                            )
                    nc.vector.tensor_copy(
                        out=dest[:, nh * 512:(nh + 1) * 512], in_=ps[:])
                    if nh == 1:
                        del inflight[key]
                        qkt_done[key] = dest
                        return dest
                    return None

                # ---------- V' chunk (j, nh): 8 bf16 matmuls ----------
                def v_chunk(j, nh):
                    if nh == 0:
                        nc.vector.memset(
                            v_sb[j][:].rearrange("p (h x) -> p h x",
                                                 x=VW)[:, :, D_K:], 1.0)
                    ps = pq2.tile([P, 512], F32, tag="pq2", name=f"psv{j}_{nh}")
                    for k in range(DP):
                        nc.tensor.matmul(
                            ps[:],
                            lhsT=xT_sb[k][:, j * P:(j + 1) * P],
                            rhs=wv_sb[k][:, nh * 512:(nh + 1) * 512],
                            start=(k == 0), stop=(k == DP - 1),
                        )
                    hs = nh * (HEADS // 2)
                    nc.vector.tensor_copy(
                        out=v_sb[j][:].rearrange(
                            "p (h x) -> p h x",
                            x=VW)[:, hs:hs + HEADS // 2, :D_K],
                        in_=ps[:].rearrange("p (h d) -> p h d", d=D_K),
                    )

                # ---------- S^T quarter stream ----------
                with tc.tile_pool(name="stp", bufs=2, space="PSUM") as stp:
                    def st_quarter(g, pair, mt, nh, h):
                        t, q = g // TILE_Q, g % TILE_Q
                        if q == 0:
                            width = min(TILE_Q, NQ - t * TILE_Q) * 512
                            st_tiles[t] = stp.tile([P, width], F32, tag="st",
                                                   name=f"st{t}")
                            et_tiles[t] = etp.tile([P, width], BF16, tag="et",
                                                   name=f"et{t}")
                        kt = qkt_done[('k', pair)]
                        qt = qkt_done[('q', pair)]
                        nc.tensor.matmul(
                            st_tiles[t][:, q * 512:(q + 1) * 512],
                            lhsT=kt[h * D_K:(h + 1) * D_K,
                                    mt * P:(mt + 1) * P],
                            rhs=qt[h * D_K:(h + 1) * D_K,
                                   nh * 512:(nh + 1) * 512],
                            start=True, stop=True,
                            tile_position=(h * D_K, 0),
                        )
                        if q == TILE_Q - 1 or g == NQ - 1:
                            nc.scalar.activation(
                                et_tiles[t][:], st_tiles[t][:],
                                mybir.ActivationFunctionType.Exp,
                                scale=exp_scale)
                            del st_tiles[t]
                            if DEBUG and t < 3:
                                nc.sync.dma_start(dbg_et[t], et_tiles[t][:])

                    def et_slice(pair, mt, nh, h):
                        g = 32 * pair + 4 * mt + 2 * nh + h
                        t, q = g // TILE_Q, g % TILE_Q
                        return et_tiles[t][:, q * 512:(q + 1) * 512]

                    # ---------- PV + normalize ----------
                    pending_norm = {}

                    def normalize_a(p, h, stg, last):
                        hg = 2 * p + h
                        dma = nc.sync.dma_start if last else \
                            nc.gpsimd.dma_start
                        dma(rs_dram[hg:hg + 1, :], stg[D_K:VW, :])
                        rsp = small.tile([P, NP_T], F32, tag="rsp",
                                         name=f"rsp{hg}")
                        dma(rsp[:], rs_dram[hg].rearrange("(p i) -> p i", p=P))
                        pending_norm[(p, h)] = (stg, rsp, last)

                    def normalize_b(p, h):
                        if (p, h) not in pending_norm:
                            return
                        stg, rsp, last = pending_norm.pop((p, h))
                        hg = 2 * p + h
                        dma = nc.sync.dma_start if last else \
                            nc.gpsimd.dma_start
                        rspr = small.tile([P, NP_T], F32, tag="rspr",
                                          name=f"rspr{hg}")
                        nc.vector.reciprocal(rspr[:], rsp[:])
                        dma(rs2_dram[hg].rearrange("(p i) -> p i", p=P),
                            rspr[:])
                        rs_row = rs2_dram[hg:hg + 1, :]
                        rs_bc = bass.AP(tensor=rs_row.tensor,
                                        offset=rs_row.offset,
                                        ap=[[0, D_K], list(rs_row.ap)[-1]])
                        rcp = small.tile([D_K, N_TOK], F32, tag="rcp",
                                         name=f"rcp{hg}")
                        dma(rcp[:], rs_bc)
                        if h == 0:
                            nc.vector.tensor_mul(out=attnT_sb[p][0:D_K, :],
                                                 in0=stg[0:D_K, :], in1=rcp[:])
                        else:
                            tmp = small.tile([D_K, N_TOK], BF16, tag="oddtmp",
                                             name=f"oddtmp{hg}")
                            nc.vector.tensor_mul(out=tmp[:],
                                                 in0=stg[0:D_K, :], in1=rcp[:])
                            dma(attnT_sb[p][D_K:P, :], tmp[:])

                    def normalize_evict(p, h, stg, last):
                        normalize_a(p, h, stg, last)
                        if last:
                            normalize_b(p, h)

                    def pv_chunk(p, slot16, last=False):
                        """4 slots per (h, nh) chain: 2 matmuls each."""
                        h, nh = slot16 // 8, (slot16 // 4) % 2
                        hg = 2 * p + h
                        q = slot16 % 4
                        if q == 0:
                            inflight[(p, h, nh)] = ppv.tile(
                                [VW, 512], F32, tag="ppv",
                                name=f"pv{p}_{h}_{nh}")
                        pvt = inflight[(p, h, nh)]
                        for mt in range(2 * q, 2 * q + 2):
                            nc.tensor.matmul(
                                pvt[:],
                                lhsT=v_sb[mt][:, hg * VW:(hg + 1) * VW],
                                rhs=et_slice(p, mt, nh, h),
                                start=(mt == 0), stop=(mt == NP_T - 1),
                            )
                        if q == 3:
                            if nh == 0:
                                inflight[("stg", p, h)] = stgp.tile(
                                    [VW, N_TOK], F32, tag="stg",
                                    name=f"stg{hg}")
                            stg = inflight[("stg", p, h)]
                            nc.vector.tensor_copy(
                                out=stg[:, nh * 512:(nh + 1) * 512],
                                in_=pvt[:])
                            del inflight[(p, h, nh)]
                            if nh == 1:
                                normalize_evict(p, h, stg, last)
                                del inflight[("stg", p, h)]
                                if h == 1:
                                    lo = 32 * p
                                    for t in range(lo // TILE_Q):
                                        et_tiles.pop(t, None)

                    # ================= ramp: pair-0 QT/KT =================
                    for nh in range(2):
                        qkt_chunk('q', 0, nh)
                    for nh in range(2):
                        qkt_chunk('k', 0, nh)

                    if DEBUG:
                        nc.sync.dma_start(dbg_qkt[0], qkt_done[('q', 0)][:])
                        nc.sync.dma_start(dbg_qkt[1], qkt_done[('k', 0)][:])

                    # deferred bulk DMAs: xT/wv on the gpsimd queue so the
                    # sync queue serves the ramp's w8 slices first
                    for k in range(DP):
                        nc.gpsimd.dma_start(xT_sb[k][:], xT[k * P:(k + 1) * P, :])
                        nc.gpsimd.dma_start(wv_sb[k][:], wv[k * P:(k + 1) * P, :])
                    for k in range(DP):
                        nc.sync.dma_start(wout_sb[k][:], wout[k * P:(k + 1) * P, :])
                    nc.sync.dma_start(bias_bc[:], bias_in)

                    # ================= main pair loop =================
                    for p in range(NPAIRS):
                        for s in range(16):      # slot = (mt, nh)
                            mt, nh = s // 2, s % 2
                            # filler: pair p+1 QT/KT (4 chunks per pair)
                            if p + 1 < NPAIRS and s < 8 and s % 2 == 0:
                                c = s // 2
                                qkt_chunk('q' if c < 2 else 'k', p + 1, c % 2)
                            # V' chunks during pair 0 (all before PV(0)),
                            # starting at slot 4 so the xT/wv DMAs can land
                            if p == 0 and s >= 4:
                                cs = [s - 4] if s < 12 else [8 + (s - 12) * 2, 9 + (s - 12) * 2]
                                for c in cs:
                                    v_chunk(c // 2, c % 2)
                            # S^T quarters (2 concurrent row-group matmuls)
                            for h in range(2):
                                g = 32 * p + 4 * mt + 2 * nh + h
                                st_quarter(g, p, mt, nh, h)
                            # PV for pair p-1 (4 slots per chain)
                            if p > 0:
                                pv_chunk(p - 1, s)
                            # deferred normalize (recip+muls) once DMAs landed
                            if s == 4 and p >= 2:
                                normalize_b(p - 2, 1)
                            if s == 12 and p >= 1:
                                normalize_b(p - 1, 0)

                if DEBUG:
                    nc.sync.dma_start(dbg_v, v_sb[0][:])
                    nc.sync.dma_start(dbg_attnT, attnT_sb[0][:])

                # ============== tail: PV(7) + projection ==============
                # (outside the stp scope so pproj's 4 banks fit)
                with (
                    tc.tile_pool(name="ev", bufs=2) as ev,
                    tc.tile_pool(name="pproj", bufs=3,
                                 space="PSUM") as pproj,
                ):
                    normalize_b(NPAIRS - 2, 1)
                    for slot16 in range(16):
                        pv_chunk(NPAIRS - 1, slot16, last=True)
                    for jg in range(NP_T // 2):
                        pss = []
                        for dj in range(2):
                            pss.append(pproj.tile([P, DIM], F32, tag="pproj",
                                                  name=f"pso{2 * jg + dj}"))
                        # pairs 0..6 for both j-tiles first (flow while the
                        # last pair's normalize is still in flight)
                        for p in range(NPAIRS):
                            for dj in range(2):
                                j = 2 * jg + dj
                                for nh in range(2):
                                    nc.tensor.matmul(
                                        pss[dj][:, nh * 512:(nh + 1) * 512],
                                        lhsT=attnT_sb[p][:, j * P:(j + 1) * P],
                                        rhs=wout_sb[p][:,
                                                       nh * 512:(nh + 1) * 512],
                                        start=(p == 0), stop=(p == NPAIRS - 1),
                                    )
                        for dj in range(2):
                            j = 2 * jg + dj
                            o = ev.tile([P, DIM], F32, tag="out", name=f"o{j}")
                            nc.vector.tensor_add(out=o[:], in0=pss[dj][:],
                                                 in1=bias_bc[:])
                            nc.sync.dma_start(out[j * P:(j + 1) * P, :], o[:])

    nc.compile()
    return nc


_NC_CACHE = None


def _get_program():
    global _NC_CACHE
    if _NC_CACHE is None:
        _NC_CACHE = build_program()
    return _NC_CACHE


def make_in_maps(x, w_qkv, w_out, b_out):
    F8NP = ml_dtypes.float8_e4m3fn
    w_qkv = np.ascontiguousarray(w_qkv).astype(np.float32)
    wv_c = np.ascontiguousarray(w_qkv[:, 2 * DIM:]).astype(ml_dtypes.bfloat16)
    w_out_c = np.ascontiguousarray(w_out).astype(ml_dtypes.bfloat16)
    b_out_c = np.ascontiguousarray(b_out).astype(np.float32)
    common = {
        "w_v": wv_c,
        "w_out": w_out_c,
        "b_out": b_out_c,
    }
    if QK_MODE == "fp8":
        # w8: [KP, 128, 2, 2048], plane i = dim-tile (2*k2 + i)
        wqk8 = (w_qkv[:, :2 * DIM] * W8SCALE).astype(F8NP)
        common["w8"] = np.ascontiguousarray(
            wqk8.reshape(KP, 2, P, 16, P).transpose(3, 2, 0, 1, 4))
    else:
        common["w_qk"] = np.ascontiguousarray(
            w_qkv[:, :2 * DIM]).astype(ml_dtypes.bfloat16)
    in_maps = []
    for b in range(N_CORES):
        xb = np.asarray(x[b], dtype=np.float32)
        xTb = np.ascontiguousarray(xb.T)
        m = dict(common)
        m["xT"] = xTb.astype(ml_dtypes.bfloat16)
        if QK_MODE == "fp8":
            x8b = xTb.astype(F8NP)  # [dim, tok]
            m["x8"] = np.ascontiguousarray(
                x8b.reshape(KP, 2, P, N_TOK).transpose(0, 2, 1, 3))
        in_maps.append(m)
    return in_maps


def kernel(x, w_qkv, w_out, b_out):
    nc = _get_program()
    in_maps = make_in_maps(x, w_qkv, w_out, b_out)
    res = run_bass_kernel_spmd(nc, in_maps, list(range(N_CORES)))
    outs = [np.asarray(r["out"], dtype=np.float32) for r in res.results]
    return np.stack(outs, axis=0)


# revision 21
# speedup vs baseline: 1.1834x; 1.0284x over previous
"""ViT attention block (B=8, N=1024, dim=1024, heads=16, d_k=64) on 8 trn2 NeuronCores.

Sharding: data-parallel over batch (1 batch per core), weights replicated.
No collectives; each core computes its batch's full attention output.

v2 design (exp-stream centric). Per-core:
  - Q/K projections run in fp8e4 DoubleRow (K=256 per matmul): host ships
    x and w_qkv[:, :2048] as fp8 "dim-pair" tensors [128, 2, *]; w scaled by
    32 (values would be subnormal in e4m3 otherwise), compensated by folding
    1/(32*32) into the exp scale. V projection stays bf16 (fp8 V costs too
    much accuracy).
  - S^T quarters: one matmul = [128 m, 512 n] for one (mt, nh, head); the
    two heads of a pair run as concurrent 64-row-group matmuls. Quarters
    stream into ping-pong PSUM tiles of 3 quarters ([128, 1536], 3 banks x
    2 bufs) so ScalarE's exp (the critical engine, ~1.9us per tile) never
    waits on a PSUM WAR hazard: S^T for tile t+1 fills while exp reads t.
  - exp(scale*S) out of PSUM -> et bf16 in SBUF (max-subtraction skipped:
    |scale*S| <~ 2, exp is exact-safe and softmax shift-invariant).
  - V' = x @ w_v with a constant-1 column per head (65 cols) so PV yields
    softmax row-sums for free; V' matmuls are emitted lazily inside the
    exp phase (PE slack) instead of a serial prologue.
  - PV(p) trails one pair behind the exp stream; [65,512] psum chains over
    8 m-tiles; staged to stg, denominator row reshaped via DRAM for a wide
    reciprocal, broadcast back, fused into the normalize multiply.
  - final = attnT.T @ w_out + b_out in the tail.
"""

import os
import numpy as np
import ml_dtypes

import concourse.bass as bass
from concourse import bacc
import concourse.mybir as mybir
import concourse.tile as tile
from concourse.bass_utils import run_bass_kernel_spmd

P = 128
N_TOK = 1024
DIM = 1024
HEADS = 16
D_K = 64
N_CORES = 8
SCALE = D_K ** -0.5  # 0.125

NP_T = N_TOK // P   # 8 token tiles
DP = DIM // P       # 8 dim tiles
KP = DP // 2        # 4 dim-pair tiles for fp8 DoubleRow
NPAIRS = HEADS // 2  # 8 head pairs
VW = D_K + 1        # 65: V columns per head incl. ones column
W8SCALE = 32.0      # host-side w_qkv fp8 pre-scale (both q and k cols)

NQ = NPAIRS * 32    # 256 S^T quarters ([128, 512] each)
TILE_Q = 3          # quarters per st/et tile
N_ST = (NQ + TILE_Q - 1) // TILE_Q  # 86 tiles (last holds 1 quarter)

BF16 = mybir.dt.bfloat16
F8 = mybir.dt.float8e4
F32 = mybir.dt.float32
DR = mybir.MatmulPerfMode.DoubleRow

# "fp8" (default) = Q/K projection in fp8 DoubleRow; "bf16" = all-bf16
QK_MODE = os.environ.get("KERNEL_QK_MODE", "fp8")


def build_program():
    nc = bacc.Bacc("TRN2", target_bir_lowering=False, debug=False)

    xT = nc.dram_tensor("xT", [DIM, N_TOK], BF16, kind="ExternalInput").ap()
    wv = nc.dram_tensor("w_v", [DIM, DIM], BF16, kind="ExternalInput").ap()
    wout = nc.dram_tensor("w_out", [DIM, DIM], BF16, kind="ExternalInput").ap()
    bout = nc.dram_tensor("b_out", [DIM], F32, kind="ExternalInput").ap()
    if QK_MODE == "fp8":
        x8 = nc.dram_tensor("x8", [KP, P, 2, N_TOK], F8,
                            kind="ExternalInput").ap()
        w8 = nc.dram_tensor("w8", [16, P, KP, 2, P], F8,
                            kind="ExternalInput").ap()
        exp_scale = float(SCALE) / (W8SCALE * W8SCALE)
    else:
        wqk = nc.dram_tensor("w_qk", [DIM, 2 * DIM], BF16,
                             kind="ExternalInput").ap()
        exp_scale = float(SCALE)
    out = nc.dram_tensor("out", [N_TOK, DIM], F32, kind="ExternalOutput").ap()
    rs_dram = nc.dram_tensor("rs_scratch", [HEADS, N_TOK], F32).ap()
    rs2_dram = nc.dram_tensor("rs2_scratch", [HEADS, N_TOK], F32).ap()
    DEBUG = os.environ.get("KERNEL_DEBUG", "0") == "1"
    if DEBUG:
        dbg_qkt = nc.dram_tensor("dbg_qkt", [2, P, N_TOK], BF16,
                                 kind="ExternalOutput").ap()
        dbg_et = nc.dram_tensor("dbg_et", [3, P, 1536], BF16,
                                kind="ExternalOutput").ap()
        dbg_v = nc.dram_tensor("dbg_v", [P, HEADS * VW], BF16,
                               kind="ExternalOutput").ap()
        dbg_attnT = nc.dram_tensor("dbg_attnT", [P, N_TOK], BF16,
                                   kind="ExternalOutput").ap()
        dbg_stg = nc.dram_tensor("dbg_stg", [2, VW, N_TOK], F32,
                                 kind="ExternalOutput").ap()
        dbg_rcp = nc.dram_tensor("dbg_rcp", [2, D_K, N_TOK], F32,
                                 kind="ExternalOutput").ap()

    with tile.TileContext(nc) as tc:
        with (
            tc.tile_pool(name="persist", bufs=1) as persist,
            tc.tile_pool(name="qkt", bufs=5) as qktp,
            tc.tile_pool(name="etp", bufs=22) as etp,
            tc.tile_pool(name="stg", bufs=3) as stgp,
            tc.tile_pool(name="small", bufs=2) as small,
            tc.tile_pool(name="w8p", bufs=4) as w8p,
        ):
            # ---------------- persistent SBUF ----------------
            x8_sb = []
            if QK_MODE == "fp8":
                x8_engines = [nc.sync, nc.scalar, nc.sync, nc.scalar]
                for k2 in range(KP):
                    t = persist.tile([P, 2, N_TOK], F8, tag=f"x8_{k2}",
                                     name=f"x8_{k2}")
                    x8_engines[k2 % 4].dma_start(t[:], x8[k2])
                    x8_sb.append(t)
            xT_sb = []
            wv_sb = []
            for k in range(DP):
                t = persist.tile([P, N_TOK], BF16, tag=f"xT{k}", name=f"xT{k}")
                xT_sb.append(t)
                w = persist.tile([P, DIM], BF16, tag=f"wv{k}", name=f"wv{k}")
                wv_sb.append(w)
            v_sb = []
            for j in range(NP_T):
                v_sb.append(persist.tile([P, HEADS * VW], BF16, tag=f"v{j}",
                                         name=f"v{j}"))
            attnT_sb = []
            for p in range(NPAIRS):
                attnT_sb.append(persist.tile([P, N_TOK], BF16, tag=f"attnT{p}",
                                             name=f"attnT{p}"))
            wout_sb = []
            for k in range(DP):
                w = persist.tile([P, DIM], BF16, tag=f"wout{k}",
                                 name=f"wout{k}")
                wout_sb.append(w)
            bias_bc = persist.tile([P, DIM], F32, tag="bias")
            bias_in = bass.AP(tensor=bout.tensor, offset=bout.offset,
                              ap=[[0, P]] + list(bout.ap))

            st_tiles = {}   # t -> PSUM tile (ping-pong)
            et_tiles = {}   # t -> SBUF bf16 tile
            qkt_done = {}   # ('q'|'k', pair) -> finished [128,1024] bf16 tile
            inflight = {}

            with (
                tc.tile_pool(name="ppv", bufs=1, space="PSUM") as ppv,
                tc.tile_pool(name="pq2", bufs=1, space="PSUM") as pq2,
            ):
                # ---------- QKT M-tile emission (chunk = one nh half) ----
                def qkt_chunk(which, pair, nh):
                    """Emit half of the QT/KT M-tile for `pair`. Returns the
                    finished [128,1024] bf16 tile after the 2nd chunk."""
                    key = (which, pair)
                    colbase = (0 if which == 'q' else DIM) + pair * P
                    if key not in inflight:
                        inflight[key] = qktp.tile([P, N_TOK], BF16, tag="qkt",
                                                  name=f"qkt_{which}{pair}")
                    dest = inflight[key]
                    ps = pq2.tile([P, 512], F32, tag="pq2",
                                  name=f"psqk_{which}{pair}_{nh}")
                    if QK_MODE == "fp8":
                        m_idx = (0 if which == 'q' else 8) + pair
                        wkey = ("w8t", which, pair)
                        if nh == 0:
                            wt = w8p.tile([P, KP, 2, P], F8, tag="w8",
                                          name=f"w8_{which}{pair}")
                            nc.sync.dma_start(wt[:], w8[m_idx])
                            inflight[wkey] = wt
                        wt = inflight[wkey]
                        for k2 in range(KP):
                            nc.tensor.matmul(
                                ps[:],
                                lhsT=wt[:, k2, :, :],
                                rhs=x8_sb[k2][:, :, nh * 512:(nh + 1) * 512],
                                start=(k2 == 0), stop=(k2 == KP - 1),
                                perf_mode=DR,
                            )
                        if nh == 1:
                            del inflight[wkey]
                    else:
                        for k in range(DP):
                            w = w8p.tile([P, P], BF16, tag="wqk",
                                         name=f"wqk_{which}{pair}_{nh}_{k}")
                            nc.sync.dma_start(
                                w[:], wqk[k * P:(k + 1) * P,
                                          colbase:colbase + P])
                            nc.tensor.matmul(
                                ps[:],
                                lhsT=w[:],
                                rhs=xT_sb[k][:, nh * 512:(nh + 1) * 512],
                                start=(k == 0), stop=(k == DP - 1),
                            )
                    nc.vector.tensor_copy(
                        out=dest[:, nh * 512:(nh + 1) * 512], in_=ps[:])
                    if nh == 1:
                        del inflight[key]
                        qkt_done[key] = dest
                        return dest
                    return None

                # ---------- V' chunk (j, nh): 8 bf16 matmuls ----------
                def v_chunk(j, nh):
                    if nh == 0:
                        nc.vector.memset(
                            v_sb[j][:].rearrange("p (h x) -> p h x",
                                                 x=VW)[:, :, D_K:], 1.0)
                    ps = pq2.tile([P, 512], F32, tag="pq2", name=f"psv{j}_{nh}")
                    for k in range(DP):
                        nc.tensor.matmul(
                            ps[:],
                            lhsT=xT_sb[k][:, j * P:(j + 1) * P],
                            rhs=wv_sb[k][:, nh * 512:(nh + 1) * 512],
                            start=(k == 0), stop=(k == DP - 1),
                        )
                    hs = nh * (HEADS // 2)
                    nc.vector.tensor_copy(
                        out=v_sb[j][:].rearrange(
                            "p (h x) -> p h x",
                            x=VW)[:, hs:hs + HEADS // 2, :D_K],
                        in_=ps[:].rearrange("p (h d) -> p h d", d=D_K),
                    )

                # ---------- S^T quarter stream ----------
                with tc.tile_pool(name="stp", bufs=2, space="PSUM") as stp:
                    def st_quarter(g, pair, mt, nh, h):
                        t, q = g // TILE_Q, g % TILE_Q
                        if q == 0:
                            width = min(TILE_Q, NQ - t * TILE_Q) * 512
                            st_tiles[t] = stp.tile([P, width], F32, tag="st",
                                                   name=f"st{t}")
                            et_tiles[t] = etp.tile([P, width], BF16, tag="et",
                                                   name=f"et{t}")
                        kt = qkt_done[('k', pair)]
                        qt = qkt_done[('q', pair)]
                        nc.tensor.matmul(
                            st_tiles[t][:, q * 512:(q + 1) * 512],
                            lhsT=kt[h * D_K:(h + 1) * D_K,
                                    mt * P:(mt + 1) * P],
                            rhs=qt[h * D_K:(h + 1) * D_K,
                                   nh * 512:(nh + 1) * 512],
                            start=True, stop=True,
                            tile_position=(h * D_K, 0),
                        )
                        if q == TILE_Q - 1 or g == NQ - 1:
                            nc.scalar.activation(
                                et_tiles[t][:], st_tiles[t][:],
                                mybir.ActivationFunctionType.Exp,
                                scale=exp_scale)
                            del st_tiles[t]
                            if DEBUG and t < 3:
                                nc.sync.dma_start(dbg_et[t], et_tiles[t][:])

                    def et_slice(pair, mt, nh, h):
                        g = 32 * pair + 4 * mt + 2 * nh + h
                        t, q = g // TILE_Q, g % TILE_Q
                        return et_tiles[t][:, q * 512:(q + 1) * 512]

                    # ---------- PV + normalize ----------
                    pending_norm = {}

                    def normalize_a(p, h, stg, last):
                        hg = 2 * p + h
                        dma = nc.sync.dma_start if last else \
                            nc.gpsimd.dma_start
                        dma(rs_dram[hg:hg + 1, :], stg[D_K:VW, :])
                        rsp = small.tile([P, NP_T], F32, tag="rsp",
                                         name=f"rsp{hg}")
                        dma(rsp[:], rs_dram[hg].rearrange("(p i) -> p i", p=P))
                        pending_norm[(p, h)] = (stg, rsp, last)

                    def normalize_b(p, h):
                        if (p, h) not in pending_norm:
                            return
                        stg, rsp, last = pending_norm.pop((p, h))
                        hg = 2 * p + h
                        dma = nc.sync.dma_start if last else \
                            nc.gpsimd.dma_start
                        rspr = small.tile([P, NP_T], F32, tag="rspr",
                                          name=f"rspr{hg}")
                        nc.vector.reciprocal(rspr[:], rsp[:])
                        dma(rs2_dram[hg].rearrange("(p i) -> p i", p=P),
                            rspr[:])
                        rs_row = rs2_dram[hg:hg + 1, :]
                        rs_bc = bass.AP(tensor=rs_row.tensor,
                                        offset=rs_row.offset,
                                        ap=[[0, D_K], list(rs_row.ap)[-1]])
                        rcp = small.tile([D_K, N_TOK], F32, tag="rcp",
                                         name=f"rcp{hg}")
                        dma(rcp[:], rs_bc)
                        if h == 0:
                            nc.vector.tensor_mul(out=attnT_sb[p][0:D_K, :],
                                                 in0=stg[0:D_K, :], in1=rcp[:])
                        else:
                            tmp = small.tile([D_K, N_TOK], BF16, tag="oddtmp",
                                             name=f"oddtmp{hg}")
                            nc.vector.tensor_mul(out=tmp[:],
                                                 in0=stg[0:D_K, :], in1=rcp[:])
                            dma(attnT_sb[p][D_K:P, :], tmp[:])

                    def normalize_evict(p, h, stg, last):
                        normalize_a(p, h, stg, last)
                        if last:
                            normalize_b(p, h)

                    def pv_chunk(p, slot16, last=False):
                        """4 slots per (h, nh) chain: 2 matmuls each."""
                        h, nh = slot16 // 8, (slot16 // 4) % 2
                        hg = 2 * p + h
                        q = slot16 % 4
                        if q == 0:
                            inflight[(p, h, nh)] = ppv.tile(
                                [VW, 512], F32, tag="ppv",
                                name=f"pv{p}_{h}_{nh}")
                        pvt = inflight[(p, h, nh)]
                        for mt in range(2 * q, 2 * q + 2):
                            nc.tensor.matmul(
                                pvt[:],
                                lhsT=v_sb[mt][:, hg * VW:(hg + 1) * VW],
                                rhs=et_slice(p, mt, nh, h),
                                start=(mt == 0), stop=(mt == NP_T - 1),
                            )
                        if q == 3:
                            if nh == 0:
                                inflight[("stg", p, h)] = stgp.tile(
                                    [VW, N_TOK], F32, tag="stg",
                                    name=f"stg{hg}")
                            stg = inflight[("stg", p, h)]
                            nc.vector.tensor_copy(
                                out=stg[:, nh * 512:(nh + 1) * 512],
                                in_=pvt[:])
                            del inflight[(p, h, nh)]
                            if nh == 1:
                                normalize_evict(p, h, stg, last)
                                del inflight[("stg", p, h)]
                                if h == 1:
                                    lo = 32 * p
                                    for t in range(lo // TILE_Q):
                                        et_tiles.pop(t, None)

                    # ================= ramp: pair-0 QT/KT =================
                    for nh in range(2):
                        qkt_chunk('q', 0, nh)
                    for nh in range(2):
                        qkt_chunk('k', 0, nh)

                    if DEBUG:
                        nc.sync.dma_start(dbg_qkt[0], qkt_done[('q', 0)][:])
                        nc.sync.dma_start(dbg_qkt[1], qkt_done[('k', 0)][:])

                    # deferred bulk DMAs: spread xT/wv over four queues so
                    # no single DMA queue's bandwidth throttles the V ramp
                    bulk_engines = [nc.gpsimd, nc.scalar, nc.gpsimd, nc.sync]
                    for k in range(DP):
                        bulk_engines[k % 4].dma_start(
                            xT_sb[k][:], xT[k * P:(k + 1) * P, :])
                        bulk_engines[k % 4].dma_start(
                            wv_sb[k][:], wv[k * P:(k + 1) * P, :])
                    for k in range(DP):
                        nc.sync.dma_start(wout_sb[k][:], wout[k * P:(k + 1) * P, :])
                    nc.sync.dma_start(bias_bc[:], bias_in)

                    # ================= main pair loop =================
                    for p in range(NPAIRS):
                        for s in range(16):      # slot = (mt, nh)
                            mt, nh = s // 2, s % 2
                            # filler: pair p+1 QT/KT (4 chunks per pair)
                            if p + 1 < NPAIRS and s < 8 and s % 2 == 0:
                                c = s // 2
                                qkt_chunk('q' if c < 2 else 'k', p + 1, c % 2)
                            # V' chunks during pair 0 (all before PV(0)),
                            # starting at slot 4 so the xT/wv DMAs can land
                            if p == 0 and s >= 4:
                                cs = [s - 4] if s < 12 else [8 + (s - 12) * 2, 9 + (s - 12) * 2]
                                for c in cs:
                                    v_chunk(c // 2, c % 2)
                            # S^T quarters (2 concurrent row-group matmuls)
                            for h in range(2):
                                g = 32 * p + 4 * mt + 2 * nh + h
                                st_quarter(g, p, mt, nh, h)
                            # PV for pair p-1 (4 slots per chain)
                            if p > 0:
                                pv_chunk(p - 1, s)
                            # deferred normalize (recip+muls) once DMAs landed
                            if s == 4 and p >= 2:
                                normalize_b(p - 2, 1)
                            if s == 12 and p >= 1:
                                normalize_b(p - 1, 0)

                if DEBUG:
                    nc.sync.dma_start(dbg_v, v_sb[0][:])
                    nc.sync.dma_start(dbg_attnT, attnT_sb[0][:])

                # ============== tail: PV(7) + projection ==============
                # (outside the stp scope so pproj's 4 banks fit)
                with (
                    tc.tile_pool(name="ev", bufs=2) as ev,
                    tc.tile_pool(name="pproj", bufs=3,
                                 space="PSUM") as pproj,
                ):
                    normalize_b(NPAIRS - 2, 1)
                    for slot16 in range(16):
                        pv_chunk(NPAIRS - 1, slot16, last=True)
                    for jg in range(NP_T // 2):
                        pss = []
                        for dj in range(2):
                            pss.append(pproj.tile([P, DIM], F32, tag="pproj",
                                                  name=f"pso{2 * jg + dj}"))
                        # pairs 0..6 for both j-tiles first (flow while the
                        # last pair's normalize is still in flight)
                        for p in range(NPAIRS):
                            for dj in range(2):
                                j = 2 * jg + dj
                                for nh in range(2):
                                    nc.tensor.matmul(
                                        pss[dj][:, nh * 512:(nh + 1) * 512],
                                        lhsT=attnT_sb[p][:, j * P:(j + 1) * P],
                                        rhs=wout_sb[p][:,
                                                       nh * 512:(nh + 1) * 512],
                                        start=(p == 0), stop=(p == NPAIRS - 1),
                                    )
                        for dj in range(2):
                            j = 2 * jg + dj
                            o = ev.tile([P, DIM], F32, tag="out", name=f"o{j}")
                            nc.vector.tensor_add(out=o[:], in0=pss[dj][:],
                                                 in1=bias_bc[:])
                            nc.sync.dma_start(out[j * P:(j + 1) * P, :], o[:])

    nc.compile()
    return nc


_NC_CACHE = None


def _get_program():
    global _NC_CACHE
    if _NC_CACHE is None:
        _NC_CACHE = build_program()
    return _NC_CACHE


def make_in_maps(x, w_qkv, w_out, b_out):
    F8NP = ml_dtypes.float8_e4m3fn
    w_qkv = np.ascontiguousarray(w_qkv).astype(np.float32)
    wv_c = np.ascontiguousarray(w_qkv[:, 2 * DIM:]).astype(ml_dtypes.bfloat16)
    w_out_c = np.ascontiguousarray(w_out).astype(ml_dtypes.bfloat16)
    b_out_c = np.ascontiguousarray(b_out).astype(np.float32)
    common = {
        "w_v": wv_c,
        "w_out": w_out_c,
        "b_out": b_out_c,
    }
    if QK_MODE == "fp8":
        # w8: [KP, 128, 2, 2048], plane i = dim-tile (2*k2 + i)
        wqk8 = (w_qkv[:, :2 * DIM] * W8SCALE).astype(F8NP)
        common["w8"] = np.ascontiguousarray(
            wqk8.reshape(KP, 2, P, 16, P).transpose(3, 2, 0, 1, 4))
    else:
        common["w_qk"] = np.ascontiguousarray(
            w_qkv[:, :2 * DIM]).astype(ml_dtypes.bfloat16)
    in_maps = []
    for b in range(N_CORES):
        xb = np.asarray(x[b], dtype=np.float32)
        xTb = np.ascontiguousarray(xb.T)
        m = dict(common)
        m["xT"] = xTb.astype(ml_dtypes.bfloat16)
        if QK_MODE == "fp8":
            x8b = xTb.astype(F8NP)  # [dim, tok]
            m["x8"] = np.ascontiguousarray(
                x8b.reshape(KP, 2, P, N_TOK).transpose(0, 2, 1, 3))
        in_maps.append(m)
    return in_maps


def kernel(x, w_qkv, w_out, b_out):
    nc = _get_program()
    in_maps = make_in_maps(x, w_qkv, w_out, b_out)
    res = run_bass_kernel_spmd(nc, in_maps, list(range(N_CORES)))
    outs = [np.asarray(r["out"], dtype=np.float32) for r in res.results]
    return np.stack(outs, axis=0)
